# revision 1
# baseline (speedup 1.0000x reference)
"""AdaptiveRankLinear on Trainium2, 8-core data-parallel Bass/Tile kernel.

Computes  y = x + gamma * (((rmsnorm(x) * norm_weight) @ U) * (S*keep)) @ V
with keep = adaptive-rank mask from the singular-value energy of S.

Sharding: x is flattened to [8192, 4096] tokens and split into 8 shards of
1024 tokens (one per NeuronCore); U/S/V/norm_weight/gamma are tiny and
replicated (folded host-side into two small matrices).

Per-core device pipeline (per 128-token tile):
  DMA x -> ACT square+accum -> rstd -> diag(rstd)
  -> PE transpose-mode matmuls xT = x_blk.T @ diag (f32)
  -> PSUM->SBUF evacuation casts to bf16 (DVE/ACT) -> PE hT = U2.T @ xT
  -> PE delta = hT.T @ V2 (512-wide chunks) -> DVE y = x + delta -> DMA out.
"""
import ml_dtypes
import numpy as np

import concourse.bass as bass
import concourse.tile as tile
from concourse import mybir
from concourse.bass_utils import run_bass_kernel_spmd
from concourse.vector_clock import ScopedClock

# ----------------------------------------------------------------------------
# Workaround: this container's walrus accepts at most ONE sync wait per
# instruction, while Tile's sem-assigner can attach several.  Split extras
# into engine-local no-ops placed immediately before the over-waited
# instruction; same for the kernel-tail drain.
# ----------------------------------------------------------------------------
_MAXW = 1


def _split_bb_waits(nc, bb):
    insts = list(bb.instructions)
    out = []
    changed = False
    for inst in insts:
        si = inst.sync_info
        if si is not None and len(si.on_wait) > _MAXW:
            changed = True
            waits = list(si.on_wait)
            extra, keep = waits[:-_MAXW], waits[-_MAXW:]
            for k, w in enumerate(extra):
                nop = mybir.InstNoOp(name=f"{inst.name}_wsplit{k}", ins=[],
                                     outs=[])
                nop.engine = inst.engine
                nop.sync_info = mybir.SyncInfo(on_wait=[w], on_update=[])
                nc.register_instruction(nop, overwrite=True)
                out.append(nop)
            inst.sync_info = mybir.SyncInfo(on_wait=keep,
                                            on_update=list(si.on_update))
        out.append(inst)
    if changed:
        bb.instructions = out


def _patched_drain_and_barrier(self, tick_clock, wait_clock):
    for f in self.nc.m.functions:
        for bb in f.blocks:
            _split_bb_waits(self.nc, bb)

    drain_inst = self.nc.sync.drain()
    wait_clock.add_sem_waits(
        drain_inst.ins, ScopedClock({None: tick_clock.global_clock})
    )
    si = drain_inst.ins.sync_info
    if si is not None and len(si.on_wait) > _MAXW:
        waits = list(si.on_wait)
        drain_inst.ins.sync_info = mybir.SyncInfo(
            on_wait=waits[:_MAXW], on_update=list(si.on_update)
        )
        rest = waits[_MAXW:]
        for i in range(0, len(rest), _MAXW):
            nop = self.nc.sync.nop(nofuse=True, hint="drain_wait_spill")
            nop.ins.sync_info = mybir.SyncInfo(
                on_wait=rest[i:i + _MAXW], on_update=[]
            )

    self.nc.all_engine_barrier()
    assert self.sems is not None
    popped = self.nc._tile_sem_poison_stack.pop()
    assert popped is self._sem_poison
    self.nc.clear_and_free_semaphores(list(self.sems.allocated().values()))
    self.nc.all_engine_barrier()


tile.TileContext._drain_and_barrier = _patched_drain_and_barrier

# ----------------------------------------------------------------------------
# Problem constants (hardcoded; kernel.py must be self-contained).
# ----------------------------------------------------------------------------
N_CORES = 8
B, T, D = 4, 2048, 4096
TOK = B * T              # 8192
R = 16
SHARD = TOK // N_CORES   # 1024
PT = 128                 # tokens per tile
NT = SHARD // PT         # 8
KB = D // 128            # 32 contraction blocks
EPS = 1e-6
ENERGY_THRESHOLD = 0.95
F32 = mybir.dt.float32
BF16 = mybir.dt.bfloat16
NP_BF16 = ml_dtypes.bfloat16
AF = mybir.ActivationFunctionType


def build_nc():
    nc = bass.Bass("TRN2", target_bir_lowering=False, debug=False,
                   num_devices=N_CORES)
    x = nc.declare_dram_parameter("x", [SHARD, D], F32, isOutput=False)
    u = nc.declare_dram_parameter("u", [128, KB * R], BF16, isOutput=False)
    v = nc.declare_dram_parameter("v", [R, D], BF16, isOutput=False)
    eye = nc.declare_dram_parameter("eye", [PT, PT], F32, isOutput=False)
    out = nc.declare_dram_parameter("out", [SHARD, D], F32, isOutput=True)

    with tile.TileContext(nc) as tc:
        with (
            tc.tile_pool(name="singles", bufs=1) as singles,
            tc.tile_pool(name="xin", bufs=4) as xin,
            tc.tile_pool(name="xtp", bufs=3) as xtp,
            tc.tile_pool(name="yout", bufs=3) as yout,
            tc.tile_pool(name="smalls", bufs=6) as smalls,
            tc.tile_pool(name="scratch", bufs=1) as scratch,
            tc.tile_pool(name="xt_ps", bufs=3, space="PSUM") as xt_ps,
            tc.tile_pool(name="h_ps", bufs=2, space="PSUM") as h_ps,
            tc.tile_pool(name="d_ps", bufs=3, space="PSUM") as d_ps,
        ):
            x0_sb = xin.tile([PT, D], F32, tag="x_sb")
            nc.sync.dma_start(out=x0_sb[:, :D // 2], in_=x[0:PT, :D // 2])
            nc.sync.dma_start(out=x0_sb[:, D // 2:], in_=x[0:PT, D // 2:])
            u_sb = singles.tile([128, KB, R], BF16)
            nc.sync.dma_start(out=u_sb, in_=u.rearrange("p (k r) -> p k r", r=R))
            v_sb = singles.tile([R, D], BF16)
            nc.sync.dma_start(out=v_sb, in_=v[:, :])
            eye_sb = singles.tile([PT, PT], F32)
            nc.sync.dma_start(out=eye_sb, in_=eye[:, :])
            eps_sb = singles.tile([128, 1], F32)
            nc.vector.memset(eps_sb, EPS)

            for it in range(NT):
                t0 = it * PT
                if it == 0:
                    x_sb = x0_sb
                else:
                    x_sb = xin.tile([PT, D], F32, tag="x_sb")
                    nc.sync.dma_start(out=x_sb, in_=x[t0:t0 + PT, :])

                # RMS stats: sumsq -> rstd
                sumsq = smalls.tile([PT, 1], F32, tag="sumsq")
                sq_scr = scratch.tile([PT, D], F32, tag="sq_scr")
                nc.scalar.activation(out=sq_scr, in_=x_sb, func=AF.Square,
                                     accum_out=sumsq)
                std = smalls.tile([PT, 1], F32, tag="std")
                nc.scalar.activation(out=std, in_=sumsq, func=AF.Sqrt,
                                     bias=eps_sb, scale=1.0 / D)
                rstd = smalls.tile([PT, 1], F32, tag="rstd")
                nc.vector.reciprocal(out=rstd, in_=std)

                # diag(rstd): eye row p is e_p -> scaled by rstd[p]
                diag = smalls.tile([PT, PT], F32, tag="diag")
                nc.vector.tensor_scalar_mul(out=diag, in0=eye_sb, scalar1=rstd)

                # PE transpose of x (scaled by rstd): xT = x_blk.T @ diag
                xt_sb = xtp.tile([128, KB * PT], BF16)
                GP = 4
                for g in range(KB // GP):
                    tp = xt_ps.tile([128, GP * PT], F32, tag="tp")
                    for q in range(GP):
                        j = g * GP + q
                        nc.tensor.matmul(
                            out=tp[:, q * PT:(q + 1) * PT],
                            lhsT=x_sb[:, j * 128:(j + 1) * 128],
                            rhs=diag, is_transpose=True,
                            start=True, stop=True)
                    dst = xt_sb[:, g * GP * PT:(g + 1) * GP * PT]
                    if g in (0, 3, 6):
                        nc.vector.tensor_copy(out=dst, in_=tp)
                    else:
                        nc.scalar.copy(out=dst, in_=tp)

                # hT = U2.T @ xT, accumulated over 32 k-blocks
                h_psum = h_ps.tile([R, PT], F32, tag="h")
                for j in range(KB):
                    nc.tensor.matmul(out=h_psum,
                                     lhsT=u_sb[:, j, :],
                                     rhs=xt_sb[:, j * PT:(j + 1) * PT],
                                     start=(j == 0), stop=(j == KB - 1))
                h_sb = smalls.tile([R, PT], BF16, tag="h_sb")
                nc.vector.tensor_copy(out=h_sb, in_=h_psum)

                # delta = hT.T @ V2 ; y = x + delta
                y_sb = yout.tile([PT, D], F32)
                for n in range(8):
                    dps = d_ps.tile([PT, 512], F32, tag="d")
                    nc.tensor.matmul(out=dps, lhsT=h_sb,
                                     rhs=v_sb[:, n * 512:(n + 1) * 512],
                                     start=True, stop=True)
                    nc.vector.tensor_add(out=y_sb[:, n * 512:(n + 1) * 512],
                                         in0=x_sb[:, n * 512:(n + 1) * 512],
                                         in1=dps)
                    if n == 3:
                        nc.gpsimd.dma_start(out=out[t0:t0 + PT, :D // 2],
                                            in_=y_sb[:, :D // 2])
                nc.gpsimd.dma_start(out=out[t0:t0 + PT, D // 2:],
                                    in_=y_sb[:, D // 2:])
    return nc


def _rank_mask_np(S):
    s_abs = np.abs(S)
    cum = np.cumsum(s_abs) / max(float(s_abs.sum()), 1e-8)
    hit = cum >= ENERGY_THRESHOLD
    r = int(np.argmax(hit)) + 1 if hit.any() else S.shape[0]
    return (np.arange(S.shape[0]) < r).astype(S.dtype)


def make_in_maps(x, U, S, V, norm_weight, gamma):
    S = np.asarray(S, dtype=np.float32)
    keep = _rank_mask_np(S)
    U2 = (np.asarray(norm_weight, dtype=np.float32)[:, None]
          * np.asarray(U, dtype=np.float32)
          * (S * keep)[None, :]).astype(NP_BF16)
    U2 = np.ascontiguousarray(
        U2.reshape(KB, 128, R).transpose(1, 0, 2).reshape(128, KB * R))
    V2 = (np.asarray(V, dtype=np.float32)
          * np.asarray(gamma, dtype=np.float32)[None, :]).astype(NP_BF16)
    eye = np.eye(PT, dtype=np.float32)
    xf = np.ascontiguousarray(np.asarray(x, dtype=np.float32).reshape(TOK, D))
    shards = np.split(xf, N_CORES, axis=0)
    return [{"x": s, "u": U2, "v": V2, "eye": eye} for s in shards]


_CACHED_NC = None


def run(x, U, S, V, norm_weight, gamma, trace=False, **kw):
    global _CACHED_NC
    if _CACHED_NC is None:
        _CACHED_NC = build_nc()
    in_maps = make_in_maps(x, U, S, V, norm_weight, gamma)
    res = run_bass_kernel_spmd(_CACHED_NC, in_maps,
                               core_ids=list(range(N_CORES)), trace=trace,
                               **kw)
    outs = [np.asarray(res.results[i]["out"]) for i in range(N_CORES)]
    y = np.concatenate(outs, axis=0).reshape(B, T, D).astype(np.float32)
    return y, res


def kernel(x, U, S, V, norm_weight, gamma):
    y, _ = run(x, U, S, V, norm_weight, gamma, trace=False)
    return y



# revision 7
# speedup vs baseline: 1.3068x; 1.3068x over previous
"""AdaptiveRankLinear on Trainium2, 8-core data-parallel Bass/Tile kernel.

Computes  y = x + gamma * (((rmsnorm(x) * norm_weight) @ U) * (S*keep)) @ V
with keep = adaptive-rank mask from the singular-value energy of S.

Sharding: x is flattened to [8192, 4096] tokens and split into 8 shards of
1024 tokens (one per NeuronCore); U/S/V/norm_weight/gamma are tiny and
replicated (folded host-side into two small matrices).

v2: bf16 I/O (halves HBM traffic -> ~47us roofline), rstd deferred to the
final fused (delta*rstd)+x op, hT accumulated directly via U-as-weights
matmuls, bf16 PSUM outputs for transposes and V-expansion.

Per-core device pipeline (per 128-token tile):
  DMA x(bf16) -> ACT square+accum -> sqrt -> DVE reciprocal -> rstd
  PE transpose-mode x_j.T @ eye (bf16 psum) -> ACT/DVE evac to SBUF
  PE hT += U2_j.T @ xT_j (f32 psum) -> DVE evac bf16
  PE delta = hT.T @ V2 (bf16 psum, 1024-wide) -> DVE y = delta*rstd + x
  -> DMA out (bf16).
"""
import ml_dtypes
import numpy as np

import concourse.bass as bass
import concourse.tile as tile
from concourse import mybir
from concourse.bass_utils import run_bass_kernel_spmd
from concourse.vector_clock import ScopedClock

# ----------------------------------------------------------------------------
# Workaround: this container's walrus accepts at most ONE sync wait per
# instruction, while Tile's sem-assigner can attach several.  Split extras
# into engine-local no-ops placed immediately before the over-waited
# instruction; same for the kernel-tail drain.
# ----------------------------------------------------------------------------
_MAXW = 1


def _split_bb_waits(nc, bb):
    insts = list(bb.instructions)
    out = []
    changed = False
    for inst in insts:
        si = inst.sync_info
        if si is not None and len(si.on_wait) > _MAXW:
            changed = True
            waits = list(si.on_wait)
            extra, keep = waits[:-_MAXW], waits[-_MAXW:]
            for k, w in enumerate(extra):
                nop = mybir.InstNoOp(name=f"{inst.name}_wsplit{k}", ins=[],
                                     outs=[])
                nop.engine = inst.engine
                nop.sync_info = mybir.SyncInfo(on_wait=[w], on_update=[])
                nc.register_instruction(nop, overwrite=True)
                out.append(nop)
            inst.sync_info = mybir.SyncInfo(on_wait=keep,
                                            on_update=list(si.on_update))
        out.append(inst)
    if changed:
        bb.instructions = out


def _patched_drain_and_barrier(self, tick_clock, wait_clock):
    for f in self.nc.m.functions:
        for bb in f.blocks:
            _split_bb_waits(self.nc, bb)

    drain_inst = self.nc.sync.drain()
    wait_clock.add_sem_waits(
        drain_inst.ins, ScopedClock({None: tick_clock.global_clock})
    )
    si = drain_inst.ins.sync_info
    if si is not None and len(si.on_wait) > _MAXW:
        waits = list(si.on_wait)
        drain_inst.ins.sync_info = mybir.SyncInfo(
            on_wait=waits[:_MAXW], on_update=list(si.on_update)
        )
        rest = waits[_MAXW:]
        for i in range(0, len(rest), _MAXW):
            nop = self.nc.sync.nop(nofuse=True, hint="drain_wait_spill")
            nop.ins.sync_info = mybir.SyncInfo(
                on_wait=rest[i:i + _MAXW], on_update=[]
            )

    self.nc.all_engine_barrier()
    assert self.sems is not None
    popped = self.nc._tile_sem_poison_stack.pop()
    assert popped is self._sem_poison
    self.nc.clear_and_free_semaphores(list(self.sems.allocated().values()))
    self.nc.all_engine_barrier()


tile.TileContext._drain_and_barrier = _patched_drain_and_barrier

# ----------------------------------------------------------------------------
# Problem constants (hardcoded; kernel.py must be self-contained).
# ----------------------------------------------------------------------------
N_CORES = 8
B, T, D = 4, 2048, 4096
TOK = B * T              # 8192
R = 16
SHARD = TOK // N_CORES   # 1024
PT = 128                 # tokens per tile
NT = SHARD // PT         # 8
KB = D // 128            # 32 contraction blocks
EPS = 1e-6
ENERGY_THRESHOLD = 0.95
F32 = mybir.dt.float32
BF16 = mybir.dt.bfloat16
NP_BF16 = ml_dtypes.bfloat16
AF = mybir.ActivationFunctionType
ALU = mybir.AluOpType

TG = 8                   # transpose blocks per PSUM group (bank = 1024 bf16)
NG = KB // TG            # 4 groups per tile
VW = 512                 # V-expansion matmul width (one f32 PSUM bank)
NV = D // VW             # 8 V-matmuls per tile


def build_nc():
    nc = bass.Bass("TRN2", target_bir_lowering=False, debug=False,
                   num_devices=N_CORES)
    x = nc.declare_dram_parameter("x", [SHARD, D], BF16, isOutput=False)
    u = nc.declare_dram_parameter("u", [128, KB * R], BF16, isOutput=False)
    v = nc.declare_dram_parameter("v", [R, D], BF16, isOutput=False)
    eye = nc.declare_dram_parameter("eye", [PT, PT], BF16, isOutput=False)
    out = nc.declare_dram_parameter("out", [SHARD, D], BF16, isOutput=True)

    with tile.TileContext(nc) as tc:
        with (
            tc.tile_pool(name="singles", bufs=1) as singles,
            tc.tile_pool(name="xin", bufs=4) as xin,
            tc.tile_pool(name="xtp", bufs=3) as xtp,
            tc.tile_pool(name="yout", bufs=3) as yout,
            tc.tile_pool(name="smalls", bufs=6) as smalls,
            tc.tile_pool(name="scratch", bufs=1) as scratch,
            tc.tile_pool(name="xt_ps", bufs=3, space="PSUM") as xt_ps,
            tc.tile_pool(name="h_ps", bufs=2, space="PSUM") as h_ps,
            tc.tile_pool(name="d_ps", bufs=3, space="PSUM") as d_ps,
        ):
            x0_sb = xin.tile([PT, D], BF16, tag="x_sb")
            nc.sync.dma_start(out=x0_sb, in_=x[0:PT, :])
            u_sb = singles.tile([128, KB, R], BF16)
            nc.sync.dma_start(out=u_sb, in_=u.rearrange("p (k r) -> p k r", r=R))
            v_sb = singles.tile([R, D], BF16)
            nc.sync.dma_start(out=v_sb, in_=v[:, :])
            eye_sb = singles.tile([PT, PT], BF16)
            nc.sync.dma_start(out=eye_sb, in_=eye[:, :])
            eps_sb = singles.tile([128, 1], F32)
            nc.vector.memset(eps_sb, EPS)

            for it in range(NT):
                t0 = it * PT
                if it == 0:
                    x_sb = x0_sb
                else:
                    x_sb = xin.tile([PT, D], BF16, tag="x_sb")
                    nc.sync.dma_start(out=x_sb, in_=x[t0:t0 + PT, :])

                # RMS stats: sumsq -> rstd (fp32 throughout)
                sumsq = smalls.tile([PT, 1], F32, tag="sumsq")
                sq_scr = scratch.tile([PT, D], BF16, tag="sq_scr")
                nc.scalar.activation(out=sq_scr, in_=x_sb, func=AF.Square,
                                     accum_out=sumsq)
                std = smalls.tile([PT, 1], F32, tag="std")
                nc.scalar.activation(out=std, in_=sumsq, func=AF.Sqrt,
                                     bias=eps_sb, scale=1.0 / D)
                rstd = smalls.tile([PT, 1], F32, tag="rstd")
                nc.vector.reciprocal(out=rstd, in_=std)

                # PE transpose (plain, unnormalized): xT_j = x_j.T @ I
                # grouped TG blocks per PSUM bank (f32), then evac to SBUF.
                xt_sb = xtp.tile([128, KB * PT], BF16)
                h_psum = h_ps.tile([R, PT], F32, tag="h")
                for g in range(NG):
                    tp = xt_ps.tile([128, TG * PT], BF16, tag="tp")
                    for q in range(TG):
                        j = g * TG + q
                        nc.tensor.matmul(
                            out=tp[:, q * PT:(q + 1) * PT],
                            lhsT=x_sb[:, j * 128:(j + 1) * 128],
                            rhs=eye_sb, is_transpose=True,
                            start=True, stop=True)
                    dst = xt_sb[:, g * TG * PT:(g + 1) * TG * PT]
                    nc.scalar.copy(out=dst, in_=tp)

                    # hT += U2_g.T @ xT_g for the TG blocks of this group
                    for q in range(TG):
                        j = g * TG + q
                        nc.tensor.matmul(
                            out=h_psum,
                            lhsT=u_sb[:, j, :],
                            rhs=xt_sb[:, j * PT:(j + 1) * PT],
                            start=(j == 0), stop=(j == KB - 1))

                h_sb = smalls.tile([R, PT], BF16, tag="h_sb")
                nc.vector.tensor_copy(out=h_sb, in_=h_psum)

                # delta = hT.T @ V2 (f32 psum, VW-wide) ; y = delta*rstd + x
                y_sb = yout.tile([PT, D], BF16)
                for n in range(NV):
                    dps = d_ps.tile([PT, VW], F32, tag="d")
                    nc.tensor.matmul(out=dps, lhsT=h_sb,
                                     rhs=v_sb[:, n * VW:(n + 1) * VW],
                                     start=True, stop=True)
                    nc.vector.scalar_tensor_tensor(
                        out=y_sb[:, n * VW:(n + 1) * VW],
                        in0=dps, scalar=rstd,
                        in1=x_sb[:, n * VW:(n + 1) * VW],
                        op0=ALU.mult, op1=ALU.add)
                    if n == NV - 2:
                        nc.gpsimd.dma_start(out=out[t0:t0 + PT, :D - VW],
                                            in_=y_sb[:, :D - VW])
                nc.gpsimd.dma_start(out=out[t0:t0 + PT, D - VW:],
                                    in_=y_sb[:, D - VW:])
    return nc


def _rank_mask_np(S):
    s_abs = np.abs(S)
    cum = np.cumsum(s_abs) / max(float(s_abs.sum()), 1e-8)
    hit = cum >= ENERGY_THRESHOLD
    r = int(np.argmax(hit)) + 1 if hit.any() else S.shape[0]
    return (np.arange(S.shape[0]) < r).astype(S.dtype)


def make_in_maps(x, U, S, V, norm_weight, gamma):
    S = np.asarray(S, dtype=np.float32)
    keep = _rank_mask_np(S)
    U2 = (np.asarray(norm_weight, dtype=np.float32)[:, None]
          * np.asarray(U, dtype=np.float32)
          * (S * keep)[None, :]).astype(NP_BF16)
    U2 = np.ascontiguousarray(
        U2.reshape(KB, 128, R).transpose(1, 0, 2).reshape(128, KB * R))
    V2 = (np.asarray(V, dtype=np.float32)
          * np.asarray(gamma, dtype=np.float32)[None, :]).astype(NP_BF16)
    eye = np.eye(PT, dtype=NP_BF16)
    xf = np.ascontiguousarray(
        np.asarray(x, dtype=np.float32).reshape(TOK, D)).astype(NP_BF16)
    shards = np.split(xf, N_CORES, axis=0)
    return [{"x": s, "u": U2, "v": V2, "eye": eye} for s in shards]


_CACHED_NC = None


def run(x, U, S, V, norm_weight, gamma, trace=False, **kw):
    global _CACHED_NC
    if _CACHED_NC is None:
        _CACHED_NC = build_nc()
    in_maps = make_in_maps(x, U, S, V, norm_weight, gamma)
    res = run_bass_kernel_spmd(_CACHED_NC, in_maps,
                               core_ids=list(range(N_CORES)), trace=trace,
                               **kw)
    outs = [np.asarray(res.results[i]["out"]) for i in range(N_CORES)]
    y = np.concatenate(outs, axis=0).reshape(B, T, D).astype(np.float32)
    return y, res


def kernel(x, U, S, V, norm_weight, gamma):
    y, _ = run(x, U, S, V, norm_weight, gamma, trace=False)
    return y
